# revision 5
# baseline (speedup 1.0000x reference)
"""Trainium2 Bass kernel for nn_CRF_70239895159020 — XBAR-transpose schedule.

Reference (B=524288, C=70, 10 iterations):
    L = (S + S^T)/2 ; dL = diag(L) ; Q = log_softmax(logits, axis=1)
    repeat 10x:  P = 2*exp(Q)-1
                 tmp = logits + P @ L - dL*P       (L symmetric)
                 Q = log_sigmoid(2*tmp)

Reformulation (M = L with zero diagonal, m2 = 2M, c = colsum(M)):
    psum_t = logits + E_t @ m2   (PE: fp16 m2 matmul + fp16 identity
                                  matmul streaming logits^T)
    tmp2_t = 2*psum_t - 2c ;  E_{t+1} = sigmoid(tmp2_t),
    E_0 = softmax(logits);  out = log_sigmoid(tmp2_9) = min(-ln(1+u), tmp2_9)
    with u = exp(-tmp2_9) (bf16, clamped to 1e15 for Ln's domain; the min
    recovers the clamped region exactly, where log_sigmoid(x) = x to fp16).

The map is chaotic (truncating even one iteration or changing the init
moves the output by O(1)), so the math is kept exactly as above; the
gains over the previous kernel are architectural:

  * All transposes moved off the PE/PSUM onto the DMA XBAR
    (InstDmaTransposeAnt, 16x128 tiles @ 14 ns): E0^T is one xbar per
    chunk from the fp16 natural softmax tile, the natural-layout logits
    are one xbar per chunk from the transposed fp16 logits (so logits
    are DMA'd once, fp16, transposed only), and the output q^T is four
    quarter xbars per chunk.  This frees the full 16 KB of PSUM for two
    [70, 2048] f32 iteration slots, which removes the psum-slot
    contention between ride transposes and iteration blocks that caused
    the old kernel's per-iteration ACT stalls (~2.1 us x 5 per chunk).
  * Output written by a gpsimd (SWDGE) casting DMA f16->f32 straight
    from the xbar staging tile: no DVE copy, no f32 staging tile.
  * fp16 logits everywhere (float64 host study: 5.86e-3 vs reference,
    gate 2e-2; the previous f32r kernel measured 5.75e-3).
  * ACT table loads pinned to 2/chunk: the sigmoid run uses
    sigmoid_and_others; exp(tmp2_9), ln and the softmax exp of chunk
    k+2 all share one natural_log_exp_and_others residency at the
    chunk boundary.

ACT is the bottleneck: 10 psum passes (9 sigmoid + 1 exp, 4x2048-col
instrs) + 1 ln + 1 softmax-exp + 2 table loads = 90.5 us/chunk, 8
chunks -> ~724 us floor; PE (matmuls only, no transposes) 68 us/chunk.

Sharding: batch split 8 ways across cores (pure data parallel).
"""

import os
import numpy as np

B = 524288
C = 70
N_CORES = 8
B_CORE = B // N_CORES
ITERS = 10

NCH = 8192            # batch columns per chunk (transposed free dim)
BLK = 2048            # psum block columns (4 banks)
CP = 80               # q16f/l1sb partition pad (xbar needs %16)

_prog_cache = {}
LAST_RESULTS = None


def build_program(b_core=B_CORE, nch=NCH, blk=BLK):
    import concourse.bass as bass
    import concourse.bacc as bacc
    import concourse.tile as tile
    from concourse import mybir
    from contextlib import ExitStack

    f32 = mybir.dt.float32
    f16 = mybir.dt.float16
    bf16 = mybir.dt.bfloat16
    AF = mybir.ActivationFunctionType
    Alu = mybir.AluOpType

    assert b_core % nch == 0
    nchunks = b_core // nch
    tpc = nch // 128
    assert nch % blk == 0
    nblk = nch // blk
    tpb = blk // 128          # t-slices per output quarter

    class _Bacc(bacc.Bacc):
        # Restrict which tables may serve the transcendentals so the
        # auto-inserter converges to 2 loads/chunk: sigmoid_and_others for
        # the sigmoid runs, natural_log_exp_and_others for the boundary
        # exp/ln/softmax-exp cluster.
        def insert_act_table_loads(self):
            from concourse.hw_specs import get_activation_tables
            has_act = any(isinstance(i, mybir.InstActivation)
                          for b in self.main_func.blocks
                          for i in b.instructions)
            if not has_act:
                return
            tabs = get_activation_tables(self.m.arch)
            AFt = mybir.ActivationFunctionType
            strip = {AFt.Exp, AFt.Ln, AFt.Sigmoid, AFt.Tanh}
            keep = ("sigmoid_and_others", "natural_log_exp_and_others")
            tables = [(n, (fs if n in keep else fs - strip))
                      for n, fs in tabs.items()]
            import bass_rust as _br
            _br.insert_act_table_loads(self, tables)

    nc = _Bacc("TRN2", target_bir_lowering=False)

    l1t_d = nc.dram_tensor("l1t", [C, b_core], f16, kind="ExternalInput")
    cf32_d = nc.dram_tensor("cf32", [C, 2], f32, kind="ExternalInput")
    cf16_d = nc.dram_tensor("cf16", [C, 2 * C], f16, kind="ExternalInput")
    out_d = nc.dram_tensor("out", [b_core, C], f32, kind="ExternalOutput")

    # natural row = k*nch + p*tpc + t ; transposed column n = t*128 + p
    # (the xbar maps [c, t*128+p] <-> [p, t, c]); l1t is permuted on the
    # host to match.
    og = out_d[:, :].rearrange("(k p t) c -> k p t c", p=128, t=tpc)

    with tile.TileContext(nc) as tc, ExitStack() as ctx:
        const = ctx.enter_context(tc.tile_pool(name="const", bufs=1))
        state = ctx.enter_context(tc.tile_pool(name="state", bufs=1))
        smallp = ctx.enter_context(tc.tile_pool(name="small", bufs=2))
        stgp = ctx.enter_context(tc.tile_pool(name="stg", bufs=2))
        psp = ctx.enter_context(tc.tile_pool(name="ps", bufs=2, space="PSUM"))

        cf32 = const.tile([C, 2], f32)
        nc.sync.dma_start(out=cf32, in_=cf32_d[:, :])
        cf16 = const.tile([C, 2 * C], f16)
        nc.sync.dma_start(out=cf16, in_=cf16_d[:, :])
        b2sb = cf32[:, 0:1]             # -2c
        b2nsb = cf32[:, 1:2]            # +2c
        m2h = cf16[:, 0:C]              # f16 m2
        idh = cf16[:, C:2 * C]          # f16 identity(70)

        # Persistent state tiles (parity pairs where two chunks overlap).
        l1sbP = [state.tile([CP, nch], f16, name=f"l1sb{i}") for i in (0, 1)]
        ek16P = [state.tile([128, nch], f16, name=f"ek16{i}") for i in (0, 1)]
        nat16P = [state.tile([128, tpc, 128], f16, name=f"nat16{i}")
                  for i in (0, 1)]
        l1xb = state.tile([128, tpc, CP], f16)
        natf = state.tile([128, tpc, C], f32)
        u16b = state.tile([C, nch], bf16)
        xn16 = state.tile([C, nch], f16)
        q16f = state.tile([CP, nch], f16)

        # Zero the xbar pad lanes once.  Engine ops must start on a
        # partition multiple of 32, so pad-row memsets cover [64:80); the
        # real rows 64..69 are rewritten by the per-chunk DMA / ln pass
        # before anything reads them.
        for i in (0, 1):
            nc.vector.memset(l1sbP[i][64:CP, :], 0.0)
            nc.vector.memset(nat16P[i][:, :, C:128], 0.0)
        nc.vector.memset(q16f[64:CP, :], 0.0)
        tc.strict_bb_all_engine_barrier()

        def dma_l1(k):
            nc.sync.dma_start(out=l1sbP[k % 2][0:C, :],
                              in_=l1t_d[:, k * nch:(k + 1) * nch])

        def xbar_l1(k):
            # natural fp16 logits for chunk k: [p, t, c] = l1sb[c, t*128+p]
            nc.sync.dma_start(out=l1xb, in_=l1sbP[k % 2][:, :],
                              transpose=True)

        def softmax(k, lo_g=0, n_g=None):
            # E0 = softmax(logits) for chunk k, natural layout, fp16 out.
            # g-granular slicing is used at startup to cut the fill bubble.
            sl = slice(lo_g, lo_g + n_g) if n_g is not None else slice(0, tpc)
            natg = natf[:, sl, :]
            nc.scalar.activation(natg, l1xb[:, sl, 0:C], AF.Exp)
            n = natg.shape[1]
            s_t = smallp.tile([128, n], f32, tag="s")
            nc.vector.reduce_sum(out=s_t, in_=natg, axis=mybir.AxisListType.X)
            r_t = smallp.tile([128, n], f32, tag="r")
            nc.vector.reciprocal(out=r_t, in_=s_t)
            t1 = smallp.tile([128, n], f32, tag="t1")
            nc.vector.tensor_mul(out=t1, in0=s_t, in1=r_t)
            nc.vector.tensor_scalar(out=t1, in0=t1, scalar1=-1.0, scalar2=2.0,
                                    op0=Alu.mult, op1=Alu.add)
            nc.vector.tensor_mul(out=r_t, in0=r_t, in1=t1)
            r_bcast = bass.AP(
                tensor=r_t.tensor, offset=r_t.offset,
                ap=[r_t.ap[0], r_t.ap[1], [0, C]])
            nc.vector.tensor_mul(out=nat16P[k % 2][:, sl, 0:C],
                                 in0=natg, in1=r_bcast)

        def xbar_e0(k):
            # E0^T: ek16[c, t*128+p] = nat16[p, t, c]
            nc.sync.dma_start(
                out=ek16P[k % 2][:, :].rearrange("c (t p) -> c t p", p=128),
                in_=nat16P[k % 2][:, :, :], transpose=True)

        def iter_blocks(k, it):
            ek16 = ek16P[k % 2]
            l1sb = l1sbP[k % 2]
            last = it == ITERS - 1
            for j in range(nblk):
                pt = psp.tile([C, blk], f32, tag="ps")
                for q in range(blk // 512):
                    lo = j * blk + q * 512
                    sub = pt[:, q * 512:(q + 1) * 512]
                    nc.tensor.matmul(sub, lhsT=m2h,
                                     rhs=ek16[0:C, lo:lo + 512],
                                     start=True, stop=False)
                    nc.tensor.matmul(sub, lhsT=idh,
                                     rhs=l1sb[0:C, lo:lo + 512],
                                     start=False, stop=True)
                jsl = slice(j * blk, (j + 1) * blk)
                if not last:
                    nc.scalar.activation(ek16[0:C, jsl], pt, AF.Sigmoid,
                                         bias=b2sb, scale=2.0)
                else:
                    # u = exp(-tmp2) (bf16: fp32 exponent range, never inf)
                    # and xn = tmp2 (fp16) per block
                    nc.scalar.activation(u16b[:, jsl], pt, AF.Exp,
                                         bias=b2nsb, scale=-2.0)
                    nc.vector.tensor_scalar(
                        out=xn16[:, jsl], in0=pt,
                        scalar1=2.0, scalar2=b2sb,
                        op0=Alu.mult, op1=Alu.add)

        def ln_pass(k):
            nc.vector.tensor_scalar_min(out=u16b, in0=u16b, scalar1=1e15)
            nc.scalar.activation(q16f[0:C, :], u16b, AF.Ln, bias=1.0)

        def out_final(k):
            # out = log_sigmoid(tmp2_9) = min(-ln(1+u), tmp2_9)
            nc.vector.scalar_tensor_tensor(
                out=q16f[0:C, :], in0=q16f[0:C, :], scalar=-1.0,
                in1=xn16, op0=Alu.mult, op1=Alu.min)

        def out_quarter(k, qq):
            stg = stgp.tile([128, tpb, CP], f16, tag="stg")
            nc.sync.dma_start(out=stg,
                              in_=q16f[:, qq * blk:(qq + 1) * blk],
                              transpose=True)
            nc.gpsimd.dma_start(out=og[k][:, qq * tpb:(qq + 1) * tpb, :],
                                in_=stg[:, :, 0:C])

        # ---- emission ----
        # startup: chunk 0 softmax in quarter granularity to cut the fill
        # bubble; chunk 1's softmax immediately after (both on the exp/ln
        # table before the first sigmoid run).
        dma_l1(0)
        xbar_l1(0)
        g4 = tpc // 4
        for g in range(4):
            softmax(0, lo_g=g * g4, n_g=g4)
        xbar_e0(0)
        dma_l1(1)
        xbar_l1(1)
        softmax(1)
        xbar_e0(1)
        iter_blocks(0, 0)

        for k in range(nchunks):
            for it in range(1, ITERS - 1):
                iter_blocks(k, it)
                if it == 1 and k >= 1 and k + 1 < nchunks:
                    # E0^T(k+1) xbar: nat16(k+1) was filled at boundary k-1
                    xbar_e0(k + 1)
            iter_blocks(k, ITERS - 1)
            if k + 2 < nchunks:
                dma_l1(k + 2)      # slot freed by it9's id-matmuls
                xbar_l1(k + 2)
            ln_pass(k)
            if k + 2 < nchunks:
                softmax(k + 2)     # exp shares the ln table residency
            if k + 1 < nchunks:
                iter_blocks(k + 1, 0)
            out_final(k)
            for qq in range(nblk):
                out_quarter(k, qq)

    nc.compile()
    return nc


def _host_prep(logits, similarities):
    S = np.asarray(similarities, dtype=np.float32)
    L = (S + S.T) * np.float32(0.5)
    M = L.copy()
    np.fill_diagonal(M, 0.0)
    m2 = (2.0 * M).astype(np.float32)
    col = M.astype(np.float64).sum(axis=0)
    cf32 = np.zeros((C, 2), dtype=np.float32)
    cf32[:, 0] = (-2.0 * col).astype(np.float32)
    cf32[:, 1] = (2.0 * col).astype(np.float32)
    cf16 = np.zeros((C, 2 * C), dtype=np.float16)
    cf16[:, 0:C] = m2.astype(np.float16)
    cf16[:, C:2 * C] = np.eye(C, dtype=np.float16)

    # device column n (of chunk k) holds batch row k*NCH + p*TPC + t where
    # n = t*128 + p; permute rows to device column order, then transpose so
    # each l1t row (one label c) is contiguous.
    tpc = NCH // 128

    def perm(a):
        b_all, c = a.shape
        v = a.reshape(b_all // NCH, 128, tpc, c)           # [k, p, t, c]
        v = np.ascontiguousarray(v.transpose(0, 2, 1, 3))  # [k, t, p, c]
        return v.reshape(b_all, c)

    l1t = np.ascontiguousarray(perm(logits).T).astype(np.float16)  # [C, B]
    return cf32, cf16, l1t


def kernel(logits, similarities):
    global LAST_RESULTS
    from concourse.bass_utils import run_bass_kernel_spmd

    logits = np.ascontiguousarray(np.asarray(logits), dtype=np.float32)
    cf32, cf16, l1t = _host_prep(logits, similarities)

    key = (B_CORE, NCH, BLK)
    if key not in _prog_cache:
        _prog_cache[key] = build_program()
    nc = _prog_cache[key]

    l1t_s = l1t.reshape(C, N_CORES, B_CORE)
    in_maps = []
    for i in range(N_CORES):
        m = {"l1t": np.ascontiguousarray(l1t_s[:, i, :]),
             "cf32": cf32, "cf16": cf16}
        in_maps.append(m)
    trace = os.environ.get("KERNEL_TRACE", "0") == "1"
    res = run_bass_kernel_spmd(nc, in_maps, core_ids=list(range(N_CORES)),
                               trace=trace)
    LAST_RESULTS = res
    out = np.concatenate([r["out"] for r in res.results], axis=0)
    return np.ascontiguousarray(out, dtype=np.float32)
